# revision 1
# baseline (speedup 1.0000x reference)
"""Trainium2 Bass kernel for nn_CrossLayer (4-layer cross network + BatchNorm).

Math per layer (reference):
    s   = out @ w_l            # [B] per-row dot
    out = x0 * s[:,None] + b_l + out
    out = (out - mean_B) * rsqrt(var_B + eps)   # BatchNorm1d, no affine

Key observation: BatchNorm (no affine) immediately follows the per-column
constant add of b_l, so b_l shifts only the column mean, which BN removes.
b is therefore mathematically irrelevant and is dropped entirely.

Strategy: data-parallel over the batch across 8 NeuronCores (1024 rows each).
On-chip layout is transposed ("layout B"): features D=2048 on partitions
(16 chunks of 128), batch on the free axis. Everything stays resident in SBUF
for all 4 layers:
  - per-row dot s: TensorE matmuls with a *replicated* stationary matrix
    w_rep[k,m] = w[k] for all m, so the accumulated PSUM result is s already
    broadcast across all 128 partitions.
  - update u = x0*s + out: VectorE tensor_tensor mult + tensor_tensor_reduce
    add (the reduce gives sum(u) for free).
  - sumsq: ScalarE Square activation with accum_out.
  - batch stats need a 16KB AllReduce (sum, sumsq packed [128,32]) per layer.
  - normalize: ScalarE activation Identity with per-partition scale/bias.
"""

import sys

for _p in ("/opt/trn_rl_repo",):
    if _p not in sys.path:
        sys.path.insert(0, _p)

import numpy as np

from concourse import bacc, bass, mybir, tile
from concourse import bass_utils

N_CORES = 8
B, D, L = 8192, 2048, 4
B_LOC = B // N_CORES          # 1024 rows per core
P = 128                       # partitions
NCH = D // P                  # 16 feature chunks
FREE = B_LOC                  # 1024 free elements (batch) per chunk
HALF = 512                    # fp32 matmul moving-N limit (one PSUM bank)
EPS = 1e-5
F32 = mybir.dt.float32
F32R = mybir.dt.float32r
BF16 = mybir.dt.bfloat16
N_SUM_ON_S = 4                # chunks whose batch-sum runs on ScalarE

_CACHE = {}


def _build(singleton_cc=False, stage=99):
    # stage: 1=dot only, 2=+update, 3=+square, 4=+allreduce, 99=full
    nc = bacc.Bacc(
        "TRN2", target_bir_lowering=False, debug=False, num_devices=N_CORES
    )
    xt_in = nc.dram_tensor("xt", [D, B_LOC], BF16, kind="ExternalInput")
    wc_in = nc.dram_tensor("wc", [P, L * NCH], F32, kind="ExternalInput")
    yt_out = nc.dram_tensor("yt", [D, B_LOC], F32, kind="ExternalOutput")

    if singleton_cc:
        AR_GROUPS = [[i] for i in range(N_CORES)]
    else:
        AR_GROUPS = [list(range(N_CORES))]

    with tile.TileContext(nc) as tc:
        with (
            tc.tile_pool(name="big", bufs=1) as big,
            tc.tile_pool(name="wrep", bufs=1) as wrep_pool,
            tc.tile_pool(name="s1p", bufs=2) as s1p,
            tc.tile_pool(name="tsc", bufs=3) as tsc,
            tc.tile_pool(name="stat", bufs=2) as statp,
            tc.tile_pool(name="stat2", bufs=2) as statp2,
            tc.tile_pool(name="ps", bufs=2, space="PSUM") as ps,
            tc.tile_pool(name="sqp", bufs=1, space="PSUM") as sqp,
            tc.tile_pool(name="warmp", bufs=1, space="PSUM") as warmp,
            tc.tile_pool(name="dram", bufs=1, space="DRAM") as dramp,
        ):
            X0 = big.tile([P, NCH, FREE], BF16, tag="x0")
            OUT = big.tile([P, NCH, FREE], F32, tag="out")
            wc = wrep_pool.tile([P, L * NCH], F32, tag="wc")
            ones = wrep_pool.tile([P, P], F32, tag="ones")
            w_rep = wrep_pool.tile([P, (L - 1) * NCH, P], F32, tag="wrep")
            w_rep0 = wrep_pool.tile([P, NCH, P], BF16, tag="wrep0")
            eps_t = wrep_pool.tile([P, 1], F32, tag="eps")
            nc.vector.memset(eps_t[:], EPS)

            # ---- warm up the collectives path (absorbs the ~38us comm-init
            # barrier while input DMA + layer-0 matmuls run) ----
            warm_in = dramp.tile([P, 1], F32, tag="warm_in")
            warm_out = dramp.tile([P, 1], F32, tag="warm_out")
            warm_sb = statp2.tile([P, 1], F32, tag="warm_sb")
            nc.vector.memset(warm_sb[:], 0.0)
            nc.gpsimd.dma_start(warm_in[:], warm_sb[:])
            nc.gpsimd.collective_compute(
                "AllReduce",
                mybir.AluOpType.add,
                replica_groups=AR_GROUPS,
                ins=[warm_in[:].opt()],
                outs=[warm_out[:].opt()],
            )

            # ---- load inputs ----
            # X0 is rounded to f32r in place right after each chunk's DMA:
            # the f32r matmuls require producers tagged as f32r, and the
            # rounding error (~1e-5 rel) is negligible.
            for c in range(NCH):
                nc.sync.dma_start(X0[:, c, :], xt_in[c * P : (c + 1) * P, :])
            nc.sync.dma_start(wc[:], wc_in[:])
            nc.vector.memset(ones[:], 1.0)
            # replicated stationary mats: w_rep[:, j, m] = wc[:, j] for all m
            # (emit layer 0's first so layer 0 matmuls can start early)
            for j in range(NCH):
                nc.vector.tensor_scalar(
                    w_rep0[:, j, :], ones[:], wc[:, j : j + 1], None,
                    mybir.AluOpType.mult,
                )
            for j in range((L - 1) * NCH):
                nc.vector.tensor_scalar(
                    w_rep[:, j, :].bitcast(F32R), ones[:], wc[:, NCH + j : NCH + j + 1],
                    None, mybir.AluOpType.mult,
                )

            for layer in range(L):
                src = X0 if layer == 0 else OUT
                # ---- per-row dot, broadcast across partitions ----
                # psum_s[p, r] = sum_d w[layer, d] * out[d, r]  (same for all p)
                # float32r: single-pass full-rate fp32 matmul (vs LOW_HIGH 2x)
                # h outer: half 0 of s completes early so VectorE can start
                # while TensorE runs half 1.
                psum_s = ps.tile([P, FREE], F32, tag="psum_s")
                for h in range(FREE // HALF):
                    for c in range(NCH):
                        if layer == 0:
                            lhsT = w_rep0[:, c, :]
                            rhs = src[:, c, h * HALF : (h + 1) * HALF]
                        else:
                            lhsT = w_rep[:, (layer - 1) * NCH + c, :].bitcast(F32R)
                            rhs = src[:, c, h * HALF : (h + 1) * HALF].bitcast(F32R)
                        nc.tensor.matmul(
                            psum_s[:, h * HALF : (h + 1) * HALF],
                            lhsT, rhs,
                            start=(c == 0),
                            stop=(c == NCH - 1),
                        )

                # layer-0 update reads X0 (already f32r-rounded); later
                # layers read OUT whose producer (the normalize below) wrote
                # through an f32r view, satisfying the bir verifier.

                HN = NCH // 2
                statsA = statp.tile([P, NCH], F32, tag="statsA")
                statsB = statp.tile([P, NCH], F32, tag="statsB")
                warm_ps = warmp.tile([P, 64], F32, tag="warm_ps")

                def _sum_slot(c):
                    tl = statsA if c < HN else statsB
                    return tl[:, (c % HN) : (c % HN) + 1]

                def _ssq_slot(c):
                    tl = statsA if c < HN else statsB
                    return tl[:, HN + (c % HN) : HN + (c % HN) + 1]

                if stage <= 1:
                    for c in range(NCH):
                        nc.vector.tensor_copy(
                            OUT[:, c, :].bitcast(F32R), psum_s[:]
                        )
                        if layer == L - 1:
                            nc.sync.dma_start(
                                yt_out[c * P : (c + 1) * P, :], OUT[:, c, :]
                            )
                    continue

                # copy s to SBUF in bf16 (2x DVE mode for the mult);
                # layer 0 folds the +1 from u = x0*(s+1)
                s1 = s1p.tile([P, FREE], BF16, tag="s1")
                nc.vector.tensor_scalar(
                    s1[:], psum_s[:], 1.0 if layer == 0 else 0.0, None,
                    mybir.AluOpType.add,
                )
                sum_on_s = set(range(0, NCH, NCH // N_SUM_ON_S)) if N_SUM_ON_S else set()
                for c in range(NCH):
                    if layer == 0:
                        nc.vector.tensor_tensor(
                            OUT[:, c, :].bitcast(F32R),
                            X0[:, c, :], s1[:],
                            mybir.AluOpType.mult,
                        )
                    else:
                        t = tsc.tile([P, FREE], BF16, tag="t")
                        for h in range(FREE // HALF):
                            nc.vector.tensor_tensor(
                                t[:, h * HALF : (h + 1) * HALF],
                                X0[:, c, h * HALF : (h + 1) * HALF],
                                s1[:, h * HALF : (h + 1) * HALF],
                                mybir.AluOpType.mult,
                            )
                        nc.vector.tensor_tensor(
                            OUT[:, c, :].bitcast(F32R), t[:],
                            OUT[:, c, :],
                            mybir.AluOpType.add,
                        )
                    if c % 4 == 2:
                        # dummy matmul spread through the V phase keeps the
                        # PE clock-gate warm for the next layer's dot
                        nc.tensor.matmul(
                            warm_ps[:], w_rep0[:, 0, :],
                            X0[:, c, :64],
                            start=True, stop=True,
                        )
                    if c in sum_on_s:
                        sqd = sqp.tile([P, FREE], F32, tag="sq")
                        nc.scalar.activation(
                            sqd[:], OUT[:, c, :],
                            mybir.ActivationFunctionType.Identity,
                            accum_out=_sum_slot(c),
                        )
                    else:
                        nc.vector.tensor_reduce(
                            _sum_slot(c), OUT[:, c, :],
                            mybir.AxisListType.X, mybir.AluOpType.add,
                        )

                if stage <= 2:
                    if layer == L - 1:
                        for c in range(NCH):
                            nc.sync.dma_start(
                                yt_out[c * P : (c + 1) * P, :], OUT[:, c, :]
                            )
                    continue

                # ---- sum of squares on ScalarE ----
                for c in range(NCH):
                    sq = sqp.tile([P, FREE], F32, tag="sq")
                    nc.scalar.activation(
                        sq[:], OUT[:, c, :],
                        mybir.ActivationFunctionType.Square,
                        accum_out=_ssq_slot(c),
                    )

                if stage <= 3:
                    if layer == L - 1:
                        for c in range(NCH):
                            nc.sync.dma_start(
                                yt_out[c * P : (c + 1) * P, :], OUT[:, c, :]
                            )
                    continue

                # ---- two AllReduces: half A (chunks 0-7) fires while the
                # update of chunks 8-15 is still running, hiding its latency.
                # NB: bounce buffers must be unique per collective.
                invs, nbs = [], []
                for half, stl in ((0, statsA), (1, statsB)):
                    ar_in = dramp.tile([P, NCH], F32, tag=f"ar_in{layer}_{half}")
                    ar_out = dramp.tile([P, NCH], F32, tag=f"ar_out{layer}_{half}")
                    nc.sync.dma_start(ar_in[:], stl[:])
                    nc.gpsimd.collective_compute(
                        "AllReduce",
                        mybir.AluOpType.add,
                        replica_groups=AR_GROUPS,
                        ins=[ar_in[:].opt()],
                        outs=[ar_out[:].opt()],
                    )
                    g = statp.tile([P, NCH], F32, tag=f"gstats{half}")
                    nc.sync.dma_start(g[:], ar_out[:])

                    mu = statp2.tile([P, HN], F32, tag=f"mu{half}")
                    ex2 = statp2.tile([P, HN], F32, tag=f"ex2{half}")
                    var = statp2.tile([P, HN], F32, tag=f"var{half}")
                    sd = statp2.tile([P, HN], F32, tag=f"sd{half}")
                    inv = statp2.tile([P, HN], F32, tag=f"inv{half}")
                    nb = statp2.tile([P, HN], F32, tag=f"nb{half}")
                    nc.vector.tensor_scalar(
                        mu[:], g[:, :HN], 1.0 / B, None, mybir.AluOpType.mult
                    )
                    nc.vector.tensor_scalar(
                        ex2[:], g[:, HN:], 1.0 / B, None, mybir.AluOpType.mult
                    )
                    nc.vector.tensor_tensor(
                        var[:], mu[:], mu[:], mybir.AluOpType.mult
                    )
                    nc.vector.tensor_tensor(
                        var[:], ex2[:], var[:], mybir.AluOpType.subtract
                    )
                    nc.scalar.activation(
                        sd[:], var[:], mybir.ActivationFunctionType.Sqrt,
                        bias=eps_t[:],
                    )
                    nc.vector.reciprocal(inv[:], sd[:])
                    nc.vector.tensor_tensor(
                        nb[:], mu[:], inv[:], mybir.AluOpType.mult
                    )
                    nc.vector.tensor_scalar(
                        nb[:], nb[:], -1.0, None, mybir.AluOpType.mult
                    )
                    invs.append(inv)
                    nbs.append(nb)

                # ---- normalize, split across ScalarE and VectorE ----
                # writes through an f32r view so next layer's matmul accepts it
                for c in range(NCH):
                    inv_h = invs[0] if c < HN else invs[1]
                    nb_h = nbs[0] if c < HN else nbs[1]
                    cc = c % HN
                    if c % 2 == 0:
                        nc.scalar.activation(
                            OUT[:, c, :].bitcast(F32R), OUT[:, c, :],
                            mybir.ActivationFunctionType.Identity,
                            bias=nb_h[:, cc : cc + 1],
                            scale=inv_h[:, cc : cc + 1],
                        )
                    else:
                        nc.vector.tensor_scalar(
                            OUT[:, c, :].bitcast(F32R), OUT[:, c, :],
                            inv_h[:, cc : cc + 1], nb_h[:, cc : cc + 1],
                            mybir.AluOpType.mult, mybir.AluOpType.add,
                        )
                    if layer == L - 1:
                        nc.sync.dma_start(
                            yt_out[c * P : (c + 1) * P, :], OUT[:, c, :]
                        )

    nc.compile()
    return nc


def _get_nc():
    if "nc" not in _CACHE:
        _CACHE["nc"] = _build()
    return _CACHE["nc"]


def kernel(x, w, b=None, **_ignored):
    x = np.ascontiguousarray(np.asarray(x, dtype=np.float32))
    w = np.asarray(w, dtype=np.float32)
    assert x.shape == (B, D) and w.shape == (L, D)

    # w_cols[p, i*NCH + c] = w[i, c*128 + p]
    w_cols = np.ascontiguousarray(
        w.reshape(L, NCH, P).transpose(2, 0, 1).reshape(P, L * NCH)
    )

    import ml_dtypes

    in_maps = []
    for m in range(N_CORES):
        xt = np.ascontiguousarray(
            x[m * B_LOC : (m + 1) * B_LOC, :].T.astype(ml_dtypes.bfloat16)
        )
        in_maps.append({"xt": xt, "wc": w_cols})

    nc = _get_nc()
    res = bass_utils.run_bass_kernel_spmd(
        nc, in_maps, core_ids=list(range(N_CORES))
    )

    out = np.empty((B, D), dtype=np.float32)
    for m in range(N_CORES):
        yt = res.results[m]["yt"]
        out[m * B_LOC : (m + 1) * B_LOC, :] = yt.T
    return out


if __name__ == "__main__":
    xs = np.random.randn(B, D).astype(np.float32)
    ws = np.random.randn(L, D).astype(np.float32)
    bs = np.random.randn(L, D).astype(np.float32)
    y = kernel(xs, ws, bs)
    print("kernel ran, out shape", y.shape)



# revision 15
# speedup vs baseline: 1.2641x; 1.2641x over previous
"""Trainium2 Bass kernel for nn_CrossLayer (4-layer cross network + BatchNorm).

Math per layer (reference):
    s   = out @ w_l            # [B] per-row dot
    out = x0 * s[:,None] + b_l + out
    out = (out - mean_B) * rsqrt(var_B + eps)   # BatchNorm1d, no affine

b_l only shifts column means, which BN removes -> dropped entirely.

Deferred normalization: BN is a per-feature affine out_hat = inv*(V - mu).
Instead of materializing out_hat each layer, track the UN-normalized state V
(per-feature constants dropped; BN of the next layer removes them):
    s_{l+1}  = V . (inv_l * w_{l+1}) - c0,   c0 = sum_d mu*inv*w  (scalar)
    V_{l+1}  = x0*s_{l+1} + inv_l*V_l        (per-feature scale fused in)
The per-chunk update is ONE scalar_tensor_tensor op (out=(V*inv)+t) whose
accum_out gives sum(V_new) for free; sumsq runs on ScalarE (Square+accum).
Only the final layer materializes out = inv*V - inv*mu (bf16, converted to
f32 on host).

Data-parallel over batch across 8 cores (1024 rows each). Layout B:
features D=2048 on partitions (16 chunks of 128), batch on the free axis.
All state bf16 (DVE 2x/4x perf modes); per-row dot s via TensorE matmuls
with replicated bf16 stationary. Batch stats: two 8KB AllReduces per layer
(half A fires mid-wave and hides its latency).
"""

import sys

for _p in ("/opt/trn_rl_repo",):
    if _p not in sys.path:
        sys.path.insert(0, _p)

import numpy as np

from concourse import bacc, bass, mybir, tile
from concourse import bass_utils

N_CORES = 8
B, D, L = 8192, 2048, 4
B_LOC = B // N_CORES          # 1024 rows per core
P = 128                       # partitions
NCH = D // P                  # 16 feature chunks
FREE = B_LOC                  # 1024 free elements (batch) per chunk
HALF = 512                    # PSUM bank limit for f32 moving-N
HN = NCH // 2                 # 8 chunks per stats half
EPS = 1e-5
F32 = mybir.dt.float32
BF16 = mybir.dt.bfloat16
ALU = mybir.AluOpType
ACTF = mybir.ActivationFunctionType
AXL = mybir.AxisListType
POOL_CHUNKS = set(range(10, 16))   # t-mult chunks offloaded to GpSimd

_CACHE = {}


def _build():
    nc = bacc.Bacc(
        "TRN2", target_bir_lowering=False, debug=False, num_devices=N_CORES
    )
    xt_in = nc.dram_tensor("xt", [D, B_LOC], BF16, kind="ExternalInput")
    wc_in = nc.dram_tensor("wc", [P, L * NCH], F32, kind="ExternalInput")
    yt_out = nc.dram_tensor("yt", [D, B_LOC], BF16, kind="ExternalOutput")
    AR_GROUPS = [list(range(N_CORES))]

    with tile.TileContext(nc) as tc:
        with (
            tc.tile_pool(name="big", bufs=1) as big,
            tc.tile_pool(name="wp", bufs=1) as wp,
            tc.tile_pool(name="s1p", bufs=2) as s1p,
            tc.tile_pool(name="tpv", bufs=3) as tpv,
            tc.tile_pool(name="tpp", bufs=6) as tpp,
            tc.tile_pool(name="stat", bufs=4) as statp,
            tc.tile_pool(name="stat2", bufs=24) as statp2,
            tc.tile_pool(name="invp", bufs=2) as invp,
            tc.tile_pool(name="c0p", bufs=2) as c0p,
            tc.tile_pool(name="ps", bufs=1, space="PSUM") as ps,
            tc.tile_pool(name="psc", bufs=2, space="PSUM") as psc,
            tc.tile_pool(name="sqp", bufs=1, space="PSUM") as sqp,
            tc.tile_pool(name="warmp", bufs=1, space="PSUM") as warmp,
            tc.tile_pool(name="dram", bufs=1, space="DRAM") as dramp,
        ):
            X0 = big.tile([P, NCH, FREE], BF16, tag="x0")
            V = big.tile([P, NCH, FREE], BF16, tag="v")
            wc = wp.tile([P, L * NCH], F32, tag="wc")
            ones = wp.tile([P, P], BF16, tag="ones")
            ones_mov = wp.tile([P, HALF], BF16, tag="ones_mov")
            wrep = wp.tile([P, L, NCH, P], BF16, tag="wrep")
            eps_t = wp.tile([P, 1], F32, tag="eps")
            nc.vector.memset(eps_t[:], EPS)
            nc.vector.memset(ones[:], 1.0)
            nc.vector.memset(ones_mov[:], 1.0)

            # ---- warm up the collectives path (absorbs the ~31us comm-init
            # barrier while input DMA + layer-0 work runs) ----
            warm_in = dramp.tile([P, 1], F32, tag="warm_in")
            warm_out = dramp.tile([P, 1], F32, tag="warm_out")
            warm_sb = statp2.tile([P, 1], F32, tag="warm_sb")
            nc.vector.memset(warm_sb[:], 0.0)
            nc.gpsimd.dma_start(warm_in[:], warm_sb[:])
            nc.gpsimd.collective_compute(
                "AllReduce",
                ALU.add,
                replica_groups=AR_GROUPS,
                ins=[warm_in[:].opt()],
                outs=[warm_out[:].opt()],
            )

            # ---- load inputs ----
            for c in range(NCH):
                nc.sync.dma_start(X0[:, c, :], xt_in[c * P : (c + 1) * P, :])
            nc.sync.dma_start(wc[:], wc_in[:])
            # layer-0 stationary: wrep[0][:, c, m] = wc[:, c] for all m
            for c in range(NCH):
                nc.vector.tensor_scalar(
                    wrep[:, 0, c, :], ones[:], wc[:, c : c + 1], None, ALU.mult
                )

            inv_t = [None] * L
            c0rep = None
            for layer in range(L):
                src = X0 if layer == 0 else V
                # ---- per-row dot, broadcast across partitions ----
                # layers>0 append a 17th stationary chunk of -c0 partials
                # times a moving ones vector: psum gets s - c0 directly
                psum_s = ps.tile([P, FREE], F32, tag="psum_s")
                nmm = NCH if layer == 0 else NCH + 1
                for c in range(nmm):
                    lhsT = wrep[:, layer, c, :] if c < NCH else c0rep[:]
                    for h in range(2):
                        rhs = (
                            src[:, c, h * HALF : (h + 1) * HALF]
                            if c < NCH
                            else ones_mov[:]
                        )
                        nc.tensor.matmul(
                            psum_s[:, h * HALF : (h + 1) * HALF],
                            lhsT,
                            rhs,
                            start=(c == 0),
                            stop=(c == nmm - 1),
                        )
                s1 = s1p.tile([P, FREE], BF16, tag="s1")
                # fold the +1 of u = x0*(s+1) at layer 0
                nc.vector.tensor_scalar(
                    s1[:], psum_s[:], 1.0 if layer == 0 else 0.0, None, ALU.add
                )

                statsA = statp.tile([P, 2 * HN], F32, tag="statsA")
                statsB = statp.tile([P, 2 * HN], F32, tag="statsB")
                stats = [statsA, statsB]

                def sum_slot(c):
                    return stats[c // HN][:, (c % HN) : (c % HN) + 1]

                def ssq_slot(c):
                    return stats[c // HN][:, HN + (c % HN) : HN + (c % HN) + 1]

                # pool t-mults issued up front so GpSimd churns while DVE
                # works the early chunks
                tts = {}
                if layer > 0:
                    for c in sorted(POOL_CHUNKS):
                        t = tpp.tile([P, FREE], BF16, tag="tp")
                        nc.gpsimd.tensor_tensor(
                            t[:], X0[:, c, :], s1[:], ALU.mult
                        )
                        tts[c] = t

                for c in range(NCH):
                    if layer == 0:
                        # V = (x0*1)*s1, sum(V) accumulated in one op
                        nc.vector.scalar_tensor_tensor(
                            V[:, c, :], X0[:, c, :], 1.0, s1[:],
                            ALU.mult, ALU.mult, accum_out=sum_slot(c),
                        )
                    else:
                        if c in POOL_CHUNKS:
                            t = tts[c]
                        else:
                            t = tpv.tile([P, FREE], BF16, tag="tv")
                            nc.vector.tensor_tensor(
                                t[:], X0[:, c, :], s1[:], ALU.mult
                            )
                        # V = (V*inv_prev) + t, sum(V) for free
                        nc.vector.scalar_tensor_tensor(
                            V[:, c, :], V[:, c, :],
                            inv_t[layer - 1][:, c : c + 1], t[:],
                            ALU.mult, ALU.add, accum_out=sum_slot(c),
                        )
                    sq = sqp.tile([P, FREE], F32, tag="sq")
                    nc.scalar.activation(
                        sq[:], V[:, c, :], ACTF.Square, accum_out=ssq_slot(c)
                    )
                    if c % 4 == 2 and layer < L - 1:
                        # dummy matmul keeps the PE clock-gate warm
                        wps = warmp.tile([P, 64], F32, tag="warm_ps")
                        nc.tensor.matmul(
                            wps[:], wrep[:, 0, 0, :], X0[:, 0, :64],
                            start=True, stop=True,
                        )

                # ---- two AllReduces; half A fires while chunks 8-15 still run
                invf = invp.tile([P, NCH], F32, tag="inv")
                inv_t[layer] = invf
                partials = (
                    statp2.tile([P, 2], F32, tag="part", name="part")
                    if layer < L - 1
                    else None
                )
                for half in range(2):
                    ar_in = dramp.tile([P, 2 * HN], F32, tag=f"ar_in{layer}_{half}")
                    ar_out = dramp.tile([P, 2 * HN], F32, tag=f"ar_out{layer}_{half}")
                    nc.sync.dma_start(ar_in[:], stats[half][:])
                    nc.gpsimd.collective_compute(
                        "AllReduce",
                        ALU.add,
                        replica_groups=AR_GROUPS,
                        ins=[ar_in[:].opt()],
                        outs=[ar_out[:].opt()],
                    )
                    g = statp.tile([P, 2 * HN], F32, tag=f"g{half}")
                    nc.sync.dma_start(g[:], ar_out[:])

                    cols = slice(half * HN, (half + 1) * HN)
                    inv = invf[:, cols]
                    mu = statp2.tile([P, HN], F32, tag=f"mu{half}")
                    var = statp2.tile([P, HN], F32, tag=f"var{half}")
                    sd = statp2.tile([P, HN], F32, tag=f"sd{half}")
                    tmp = statp2.tile([P, HN], F32, tag=f"mmu{half}")
                    nc.vector.tensor_scalar(
                        mu[:], g[:, :HN], 1.0 / B, None, ALU.mult
                    )
                    nc.vector.tensor_scalar(
                        var[:], g[:, HN:], 1.0 / B, None, ALU.mult
                    )
                    nc.vector.tensor_tensor(tmp[:], mu[:], mu[:], ALU.mult)
                    nc.vector.tensor_tensor(var[:], var[:], tmp[:], ALU.subtract)
                    nc.scalar.activation(
                        sd[:], var[:], ACTF.Sqrt, bias=eps_t[:]
                    )
                    nc.vector.reciprocal(inv, sd[:])

                    if layer < L - 1:
                        # next-layer scaled stationary + c0 partial
                        wsc = statp2.tile([P, HN], F32, tag=f"wsc{half}")
                        base = (layer + 1) * NCH + half * HN
                        nc.vector.tensor_tensor(
                            wsc[:], inv, wc[:, base : base + HN], ALU.mult
                        )
                        for j in range(HN):
                            c = half * HN + j
                            nc.vector.tensor_scalar(
                                wrep[:, layer + 1, c, :], ones[:],
                                wsc[:, j : j + 1], None, ALU.mult,
                            )
                        gw = statp2.tile([P, HN], F32, tag=f"gw{half}")
                        nc.vector.tensor_tensor(gw[:], mu[:], wsc[:], ALU.mult)
                        nc.vector.tensor_reduce(
                            partials[:, half : half + 1], gw[:], AXL.X, ALU.add
                        )
                    else:
                        # final materialization: out = inv*V - inv*mu
                        nbh = statp2.tile([P, HN], F32, tag=f"nb{half}")
                        nc.vector.tensor_tensor(nbh[:], mu[:], inv, ALU.mult)
                        nc.vector.tensor_scalar(
                            nbh[:], nbh[:], -1.0, None, ALU.mult
                        )
                        for j in range(HN):
                            c = half * HN + j
                            nc.vector.tensor_scalar(
                                V[:, c, :], V[:, c, :],
                                invf[:, c : c + 1], nbh[:, j : j + 1],
                                ALU.mult, ALU.add,
                            )
                            nc.sync.dma_start(
                                yt_out[c * P : (c + 1) * P, :], V[:, c, :]
                            )

                if layer < L - 1:
                    # -c0 partials replicated into a [128,128] bf16 stationary;
                    # its column sums against a ones moving vector give -c0
                    pr = statp2.tile([P, 1], F32, tag="pr")
                    nc.vector.tensor_reduce(pr[:], partials[:], AXL.X, ALU.add)
                    c0rep = c0p.tile([P, P], BF16, tag="c0rep")
                    nc.vector.tensor_scalar(
                        c0rep[:], ones[:], pr[:], -1.0, ALU.mult, ALU.mult
                    )

    nc.compile()
    return nc


def _get_nc():
    if "nc" not in _CACHE:
        _CACHE["nc"] = _build()
    return _CACHE["nc"]


def _prep_in_maps(x, w):
    import ml_dtypes

    x = np.ascontiguousarray(np.asarray(x, dtype=np.float32))
    w = np.asarray(w, dtype=np.float32)
    assert x.shape == (B, D) and w.shape == (L, D)
    # w_cols[p, i*NCH + c] = w[i, c*128 + p]
    w_cols = np.ascontiguousarray(
        w.reshape(L, NCH, P).transpose(2, 0, 1).reshape(P, L * NCH)
    )
    in_maps = []
    for m in range(N_CORES):
        xt = np.ascontiguousarray(
            x[m * B_LOC : (m + 1) * B_LOC, :].T.astype(ml_dtypes.bfloat16)
        )
        in_maps.append({"xt": xt, "wc": w_cols})
    return in_maps


def kernel(x, w, b=None, **_ignored):
    in_maps = _prep_in_maps(x, w)
    nc = _get_nc()
    res = bass_utils.run_bass_kernel_spmd(
        nc, in_maps, core_ids=list(range(N_CORES))
    )
    out = np.empty((B, D), dtype=np.float32)
    for m in range(N_CORES):
        yt = res.results[m]["yt"]
        out[m * B_LOC : (m + 1) * B_LOC, :] = yt.astype(np.float32).T
    return out


if __name__ == "__main__":
    xs = np.random.randn(B, D).astype(np.float32)
    ws = np.random.randn(L, D).astype(np.float32)
    bs = np.random.randn(L, D).astype(np.float32)
    y = kernel(xs, ws, bs)
    print("kernel ran, out shape", y.shape)


# revision 20
# speedup vs baseline: 1.2956x; 1.0249x over previous
"""Trainium2 Bass kernel for nn_CrossLayer (4-layer cross network + BatchNorm).

Math per layer (reference):
    s   = out @ w_l            # [B] per-row dot
    out = x0 * s[:,None] + b_l + out
    out = (out - mean_B) * rsqrt(var_B + eps)   # BatchNorm1d, no affine

b_l only shifts column means, which BN removes -> dropped entirely.

Deferred normalization: BN is a per-feature affine out_hat = inv*(V - mu).
Instead of materializing out_hat each layer, track the UN-normalized state V
(per-feature constants dropped; BN of the next layer removes them):
    s_{l+1}  = V . (inv_l * w_{l+1}) - c0,   c0 = sum_d mu*inv*w  (scalar)
    V_{l+1}  = x0*s_{l+1} + inv_l*V_l        (per-feature scale fused in)
The per-chunk update is ONE scalar_tensor_tensor op (out=(V*inv)+t) whose
accum_out gives sum(V_new) for free; sumsq runs on ScalarE (Square+accum).
Only the final layer materializes out = inv*V - inv*mu (bf16, converted to
f32 on host).

Data-parallel over batch across 8 cores (1024 rows each). Layout B:
features D=2048 on partitions (16 chunks of 128), batch on the free axis.
All state bf16 (DVE 2x/4x perf modes); per-row dot s via TensorE matmuls
with replicated bf16 stationary. Batch stats: two 8KB AllReduces per layer
(half A fires mid-wave and hides its latency).
"""

import sys

for _p in ("/opt/trn_rl_repo",):
    if _p not in sys.path:
        sys.path.insert(0, _p)

import numpy as np

from concourse import bacc, bass, mybir, tile
from concourse import bass_utils

N_CORES = 8
B, D, L = 8192, 2048, 4
B_LOC = B // N_CORES          # 1024 rows per core
P = 128                       # partitions
NCH = D // P                  # 16 feature chunks
FREE = B_LOC                  # 1024 free elements (batch) per chunk
HALF = 512                    # PSUM bank limit for f32 moving-N
HN = NCH // 2                 # 8 chunks per stats half
EPS = 1e-5
F32 = mybir.dt.float32
BF16 = mybir.dt.bfloat16
ALU = mybir.AluOpType
ACTF = mybir.ActivationFunctionType
AXL = mybir.AxisListType
POOL_CHUNKS = set(range(10, 16))   # t-mult chunks offloaded to GpSimd

_CACHE = {}


def _build():
    nc = bacc.Bacc(
        "TRN2", target_bir_lowering=False, debug=False, num_devices=N_CORES
    )
    xt_in = nc.dram_tensor("xt", [D, B_LOC], BF16, kind="ExternalInput")
    wc_in = nc.dram_tensor("wc", [P, L * NCH], F32, kind="ExternalInput")
    yt_out = nc.dram_tensor("yt", [D, B_LOC], BF16, kind="ExternalOutput")
    AR_GROUPS = [list(range(N_CORES))]

    with tile.TileContext(nc) as tc:
        with (
            tc.tile_pool(name="big", bufs=1) as big,
            tc.tile_pool(name="wp", bufs=1) as wp,
            tc.tile_pool(name="s1p", bufs=2) as s1p,
            tc.tile_pool(name="tpv", bufs=3) as tpv,
            tc.tile_pool(name="tpp", bufs=6) as tpp,
            tc.tile_pool(name="stat", bufs=4) as statp,
            tc.tile_pool(name="stat2", bufs=24) as statp2,
            tc.tile_pool(name="invp", bufs=2) as invp,
            tc.tile_pool(name="c0p", bufs=2) as c0p,
            tc.tile_pool(name="ps", bufs=1, space="PSUM") as ps,
            tc.tile_pool(name="psc", bufs=2, space="PSUM") as psc,
            tc.tile_pool(name="sqp", bufs=1, space="PSUM") as sqp,
            tc.tile_pool(name="warmp", bufs=1, space="PSUM") as warmp,
            tc.tile_pool(name="dram", bufs=1, space="DRAM") as dramp,
        ):
            # ---- warm up the collectives path FIRST: the comm-init stalls
            # the whole device ~20us, so trigger it as early as possible ----
            warm_in = dramp.tile([P, 1], F32, tag="warm_in")
            warm_out = dramp.tile([P, 1], F32, tag="warm_out")
            warm_sb = statp2.tile([P, 1], F32, tag="warm_sb")
            nc.gpsimd.memset(warm_sb[:], 0.0)
            nc.gpsimd.dma_start(warm_in[:], warm_sb[:])
            nc.gpsimd.collective_compute(
                "AllReduce",
                ALU.add,
                replica_groups=AR_GROUPS,
                ins=[warm_in[:].opt()],
                outs=[warm_out[:].opt()],
            )

            X0 = big.tile([P, NCH, FREE], BF16, tag="x0")
            Va = big.tile([P, NCH, FREE], BF16, tag="va")
            Vb = big.tile([P, NCH, FREE], BF16, tag="vb")
            wc = wp.tile([P, L * NCH], F32, tag="wc")
            ones = wp.tile([P, P], BF16, tag="ones")
            ones_mov = wp.tile([P, HALF], BF16, tag="ones_mov")
            wrep = wp.tile([P, L, NCH, P], BF16, tag="wrep")
            eps_t = wp.tile([P, 1], F32, tag="eps")
            nc.vector.memset(eps_t[:], EPS)
            nc.vector.memset(ones[:], 1.0)
            nc.vector.memset(ones_mov[:], 1.0)

            # ---- load inputs ----
            for c in range(NCH):
                nc.sync.dma_start(X0[:, c, :], xt_in[c * P : (c + 1) * P, :])
            nc.sync.dma_start(wc[:], wc_in[:])
            # layer-0 stationary: wrep[0][:, c, m] = wc[:, c] for all m
            for c in range(NCH):
                nc.vector.tensor_scalar(
                    wrep[:, 0, c, :], ones[:], wc[:, c : c + 1], None, ALU.mult
                )

            inv_t = [None] * L
            c0rep = None
            for layer in range(L):
                # state ping-pong: layer l reads Vprev, writes Vcur
                Vprev = Va if layer % 2 == 1 else Vb
                Vcur = Va if layer % 2 == 0 else Vb
                src = X0 if layer == 0 else Vprev
                # ---- per-row dot, broadcast across partitions ----
                # layers>0 append a 17th stationary chunk of -c0 partials
                # times a moving ones vector: psum gets s - c0 directly
                psum_s = ps.tile([P, FREE], F32, tag="psum_s")
                nmm = NCH if layer == 0 else NCH + 1
                for c in range(nmm):
                    lhsT = wrep[:, layer, c, :] if c < NCH else c0rep[:]
                    for h in range(2):
                        rhs = (
                            src[:, c, h * HALF : (h + 1) * HALF]
                            if c < NCH
                            else ones_mov[:]
                        )
                        nc.tensor.matmul(
                            psum_s[:, h * HALF : (h + 1) * HALF],
                            lhsT,
                            rhs,
                            start=(c == 0),
                            stop=(c == nmm - 1),
                        )
                s1 = s1p.tile([P, FREE], BF16, tag="s1")
                # fold the +1 of u = x0*(s+1) at layer 0
                nc.vector.tensor_scalar(
                    s1[:], psum_s[:], 1.0 if layer == 0 else 0.0, None, ALU.add
                )

                statsA = statp.tile([P, 2 * HN], F32, tag="statsA")
                statsB = statp.tile([P, 2 * HN], F32, tag="statsB")
                stats = [statsA, statsB]

                def sum_slot(c):
                    return stats[c // HN][:, (c % HN) : (c % HN) + 1]

                def ssq_slot(c):
                    return stats[c // HN][:, HN + (c % HN) : HN + (c % HN) + 1]

                # pool t-mults issued up front so GpSimd churns while DVE
                # works the early chunks
                tts = {}
                if layer > 0:
                    for c in sorted(POOL_CHUNKS):
                        t = tpp.tile([P, FREE], BF16, tag="tp")
                        nc.gpsimd.tensor_tensor(
                            t[:], X0[:, c, :], s1[:], ALU.mult
                        )
                        tts[c] = t

                for c in range(NCH):
                    if layer == 0:
                        # V = (x0*1)*s1, sum(V) accumulated in one op
                        nc.vector.scalar_tensor_tensor(
                            Vcur[:, c, :], X0[:, c, :], 1.0, s1[:],
                            ALU.mult, ALU.mult, accum_out=sum_slot(c),
                        )
                    else:
                        if c in POOL_CHUNKS:
                            t = tts[c]
                        else:
                            t = tpv.tile([P, FREE], BF16, tag="tv")
                            nc.vector.tensor_tensor(
                                t[:], X0[:, c, :], s1[:], ALU.mult
                            )
                        # V = (Vprev*inv_prev) + t, sum(V) for free
                        nc.vector.scalar_tensor_tensor(
                            Vcur[:, c, :], Vprev[:, c, :],
                            inv_t[layer - 1][:, c : c + 1], t[:],
                            ALU.mult, ALU.add, accum_out=sum_slot(c),
                        )
                    sq = sqp.tile([P, FREE], F32, tag="sq")
                    nc.scalar.activation(
                        sq[:], Vcur[:, c, :], ACTF.Square, accum_out=ssq_slot(c)
                    )
                    if c % 2 == 0 and layer < L - 1:
                        # dummy matmul keeps the PE clock-gate warm/ramped
                        wps = warmp.tile([P, 128], F32, tag="warm_ps")
                        nc.tensor.matmul(
                            wps[:], wrep[:, 0, 0, :], X0[:, 0, :128],
                            start=True, stop=True,
                        )

                # ---- two AllReduces; half A fires while chunks 8-15 still run
                invf = invp.tile([P, NCH], F32, tag="inv")
                inv_t[layer] = invf
                partials = (
                    statp2.tile([P, 2], F32, tag="part", name="part")
                    if layer < L - 1
                    else None
                )
                for half in range(2):
                    ar_in = dramp.tile([P, 2 * HN], F32, tag=f"ar_in{layer}_{half}")
                    ar_out = dramp.tile([P, 2 * HN], F32, tag=f"ar_out{layer}_{half}")
                    nc.sync.dma_start(ar_in[:], stats[half][:])
                    nc.gpsimd.collective_compute(
                        "AllReduce",
                        ALU.add,
                        replica_groups=AR_GROUPS,
                        ins=[ar_in[:].opt()],
                        outs=[ar_out[:].opt()],
                    )
                    g = statp.tile([P, 2 * HN], F32, tag=f"g{half}")
                    nc.sync.dma_start(g[:], ar_out[:])

                    cols = slice(half * HN, (half + 1) * HN)
                    inv = invf[:, cols]
                    mu = statp2.tile([P, HN], F32, tag=f"mu{half}")
                    var = statp2.tile([P, HN], F32, tag=f"var{half}")
                    sd = statp2.tile([P, HN], F32, tag=f"sd{half}")
                    tmp = statp2.tile([P, HN], F32, tag=f"mmu{half}")
                    # small stats math on GpSimd (idle between waves), sqrt on
                    # ScalarE, reciprocal on DVE (its op only)
                    nc.vector.tensor_scalar(
                        mu[:], g[:, :HN], 1.0 / B, None, ALU.mult
                    )
                    nc.vector.tensor_scalar(
                        var[:], g[:, HN:], 1.0 / B, None, ALU.mult
                    )
                    nc.vector.tensor_tensor(tmp[:], mu[:], mu[:], ALU.mult)
                    nc.vector.tensor_tensor(var[:], var[:], tmp[:], ALU.subtract)
                    nc.scalar.activation(
                        sd[:], var[:], ACTF.Sqrt, bias=eps_t[:]
                    )
                    nc.vector.reciprocal(inv, sd[:])

                    if layer < L - 1:
                        # next-layer scaled stationary + c0 partial
                        wsc = statp2.tile([P, HN], F32, tag=f"wsc{half}")
                        base = (layer + 1) * NCH + half * HN
                        nc.vector.tensor_tensor(
                            wsc[:], inv, wc[:, base : base + HN], ALU.mult
                        )
                        for j in range(HN):
                            c = half * HN + j
                            nc.vector.tensor_scalar(
                                wrep[:, layer + 1, c, :], ones[:],
                                wsc[:, j : j + 1], None, ALU.mult,
                            )
                        gw = statp2.tile([P, HN], F32, tag=f"gw{half}")
                        nc.vector.tensor_tensor(gw[:], mu[:], wsc[:], ALU.mult)
                        nc.vector.tensor_reduce(
                            partials[:, half : half + 1], gw[:], AXL.X, ALU.add
                        )
                    else:
                        # final materialization: out = inv*V - inv*mu
                        nbh = statp2.tile([P, HN], F32, tag=f"nb{half}")
                        nc.vector.tensor_tensor(nbh[:], mu[:], inv, ALU.mult)
                        nc.vector.tensor_scalar(
                            nbh[:], nbh[:], -1.0, None, ALU.mult
                        )
                        for j in range(HN):
                            c = half * HN + j
                            nc.vector.tensor_scalar(
                                Vcur[:, c, :], Vcur[:, c, :],
                                invf[:, c : c + 1], nbh[:, j : j + 1],
                                ALU.mult, ALU.add,
                            )
                            nc.sync.dma_start(
                                yt_out[c * P : (c + 1) * P, :], Vcur[:, c, :]
                            )

                if layer < L - 1:
                    # -c0 partials replicated into a [128,128] bf16 stationary;
                    # its column sums against a ones moving vector give -c0
                    pr = statp2.tile([P, 1], F32, tag="pr")
                    nc.vector.tensor_reduce(pr[:], partials[:], AXL.X, ALU.add)
                    c0rep = c0p.tile([P, P], BF16, tag="c0rep")
                    nc.vector.tensor_scalar(
                        c0rep[:], ones[:], pr[:], -1.0, ALU.mult, ALU.mult
                    )

    nc.compile()
    return nc


def _get_nc():
    if "nc" not in _CACHE:
        _CACHE["nc"] = _build()
    return _CACHE["nc"]


def _prep_in_maps(x, w):
    import ml_dtypes

    x = np.ascontiguousarray(np.asarray(x, dtype=np.float32))
    w = np.asarray(w, dtype=np.float32)
    assert x.shape == (B, D) and w.shape == (L, D)
    # w_cols[p, i*NCH + c] = w[i, c*128 + p]
    w_cols = np.ascontiguousarray(
        w.reshape(L, NCH, P).transpose(2, 0, 1).reshape(P, L * NCH)
    )
    in_maps = []
    for m in range(N_CORES):
        xt = np.ascontiguousarray(
            x[m * B_LOC : (m + 1) * B_LOC, :].T.astype(ml_dtypes.bfloat16)
        )
        in_maps.append({"xt": xt, "wc": w_cols})
    return in_maps


def kernel(x, w, b=None, **_ignored):
    in_maps = _prep_in_maps(x, w)
    nc = _get_nc()
    res = bass_utils.run_bass_kernel_spmd(
        nc, in_maps, core_ids=list(range(N_CORES))
    )
    out = np.empty((B, D), dtype=np.float32)
    for m in range(N_CORES):
        yt = res.results[m]["yt"]
        out[m * B_LOC : (m + 1) * B_LOC, :] = yt.astype(np.float32).T
    return out


if __name__ == "__main__":
    xs = np.random.randn(B, D).astype(np.float32)
    ws = np.random.randn(L, D).astype(np.float32)
    bs = np.random.randn(L, D).astype(np.float32)
    y = kernel(xs, ws, bs)
    print("kernel ran, out shape", y.shape)
